# revision 25
# baseline (speedup 1.0000x reference)
"""Trainium2 Bass kernel for ComplexGCN (3x GCNConv + 2x MHA), 8-core SPMD.

Strategy: shard destination nodes across 8 cores (512 nodes/core). Each core
builds its dense adjacency shard A^T [4096 src, 512 dest] holding RAW edge
weights (fp8) on-device from the (host-sorted) edge list via iota/is_equal
one-hot matmuls. The GCN sym-norm dinv factors are folded into activation
evictions (dinv[src] onto the aggregated rhs, dinv[dest] onto the psum
eviction), so A itself is never rescaled. Every layer's message passing is a
dense fp8 matmul with A^T stationary. Attention runs in transposed
(feature-major) fp8 layout with ACT exp and a ones-column in V for the
softmax denominator; the denominator reciprocal is computed once per MHA on
a transposed [128,16] layout (fast on DVE) and applied via a rank-1
broadcast matmul + fused tensor_tensor multiply. Cross-core: AllGather of
the degree vector and of fp8 node-feature matrices between stages, with
batched rearranged DMAs for scatter/gather around the collective.

Host-side work is limited to index manipulation / layout (sort, pad,
transpose, concat); all floating-point math on input values happens
on-device.
"""

import numpy as np

import concourse.bass as bass
import concourse.bacc as bacc
import concourse.mybir as mybir
import concourse.tile as tile
from concourse import bass_utils
from concourse.masks import make_identity

P = 128
N = 4096
NCORES = 8
NPC = N // NCORES          # 512 nodes per core
NSTRIP = NPC // P          # 4 dest strips per core
NST = N // P               # 32 src tiles
DIN = 256
HID = 256
DOUT = 128
NH = 4
DH = 64
RSCALE = 64.0              # fp8 range boost for post-MHA activations

f32 = mybir.dt.float32
bf16 = mybir.dt.bfloat16
fp8 = mybir.dt.float8e4
AF = mybir.ActivationFunctionType
ALU = mybir.AluOpType
RG = [list(range(NCORES))]


# ----------------------------------------------------------------------------
# Host-side prep: pure index manipulation + layout.
# ----------------------------------------------------------------------------

def _prep_edges(edge_index, edge_weight):
    """Partition/sort/pad edges per core into fixed chunk cells.

    Returns (M, cell_off, edat) with edat = concat(erow, ecol, eww) along
    columns, bf16, per core: [NCORES, 128, 3C]. Relative ids (pad -1),
    weights (pad 0).
    """
    rows = np.concatenate([np.asarray(edge_index[0], np.int64),
                           np.arange(N, dtype=np.int64)])
    cols = np.concatenate([np.asarray(edge_index[1], np.int64),
                           np.arange(N, dtype=np.int64)])
    w = np.concatenate([np.asarray(edge_weight, np.float32),
                        np.ones(N, np.float32)])

    core = cols // NPC
    strip = (cols % NPC) // P
    stile = rows // P
    cell = (core * NSTRIP + strip) * NST + stile
    order = np.argsort(cell, kind="stable")
    srows, scols, sw, scell = rows[order], cols[order], w[order], cell[order]

    cnt = np.bincount(cell, minlength=NCORES * NSTRIP * NST)
    cnt = cnt.reshape(NCORES, NSTRIP, NST)
    M = (-((-cnt) // P)).max(axis=0)                  # ceil, max over cores
    C = int(M.sum())
    cell_off = np.zeros((NSTRIP, NST), np.int64)
    off = 0
    for s in range(NSTRIP):
        for t in range(NST):
            cell_off[s, t] = off
            off += M[s, t]

    erow = np.full((NCORES, P, C), -1.0, np.float32)
    ecol = np.full((NCORES, P, C), -1.0, np.float32)
    eww = np.zeros((NCORES, P, C), np.float32)
    starts = np.searchsorted(scell, np.arange(NCORES * NSTRIP * NST + 1))
    for c in range(NCORES):
        for s in range(NSTRIP):
            for t in range(NST):
                k = (c * NSTRIP + s) * NST + t
                a, b = int(starts[k]), int(starts[k + 1])
                n = b - a
                if n == 0:
                    continue
                m = int(M[s, t])
                o = int(cell_off[s, t])
                rr = np.full(m * P, -1.0, np.float32)
                cc = np.full(m * P, -1.0, np.float32)
                ww = np.zeros(m * P, np.float32)
                rr[:n] = (srows[a:b] % P).astype(np.float32)
                cc[:n] = (scols[a:b] % P).astype(np.float32)
                ww[:n] = sw[a:b]
                erow[c, :, o:o + m] = rr.reshape(m, P).T
                ecol[c, :, o:o + m] = cc.reshape(m, P).T
                eww[c, :, o:o + m] = ww.reshape(m, P).T
    edat = np.concatenate([erow, ecol, eww], axis=2)
    return M, cell_off, edat


# ----------------------------------------------------------------------------
# Device program
# ----------------------------------------------------------------------------

def _build_program(M, cell_off):
    C = int(M.sum())
    nc = bacc.Bacc("TRN2", target_bir_lowering=False, debug=False,
                   num_devices=NCORES)

    # ---- external I/O ----
    d_x0 = nc.dram_tensor("x0", [N, DIN], f32, kind="ExternalInput")
    d_W1b = nc.dram_tensor("W1b", [DIN + 1, HID], f32, kind="ExternalInput")
    d_W2b = nc.dram_tensor("W2b", [HID + 1, HID], f32, kind="ExternalInput")
    d_W3b = nc.dram_tensor("W3b", [HID + 1, DOUT], f32, kind="ExternalInput")
    d_ipwT = nc.dram_tensor("ipwT", [HID, 3 * HID], f32, kind="ExternalInput")
    d_ipbC = nc.dram_tensor("ipbC", [P, 6], f32, kind="ExternalInput")
    d_opwTb = nc.dram_tensor("opwTb", [HID + 1, HID], f32, kind="ExternalInput")
    d_bvb = nc.dram_tensor("bvb", [1, HID], f32, kind="ExternalInput")
    d_edat = nc.dram_tensor("edat", [P, 3 * C], f32, kind="ExternalInput")
    d_out = nc.dram_tensor("out", [NPC, DOUT], f32, kind="ExternalOutput")

    # ---- internal DRAM for collectives ----
    d_degl = nc.dram_tensor("deg_loc", [NPC], f32)
    d_degg = nc.dram_tensor("deg_glob", [N], f32, addr_space="Shared")
    ag_bufs = []
    for i, (shape_l, shape_g) in enumerate([
        ([HID, NPC], [NCORES, HID, NPC]),      # x1T  (f-major)
        ([NPC, HID], [NCORES, NPC, HID]),      # x2   (n-major)
        ([HID, NPC], [NCORES, HID, NPC]),      # x3T  (f-major)
        ([NPC, HID], [NCORES, NPC, HID]),      # x4   (n-major)
    ]):
        loc = nc.dram_tensor(f"ag{i}_loc", shape_l, fp8)
        glob = nc.dram_tensor(f"ag{i}_glob", shape_g, fp8, addr_space="Shared")
        ag_bufs.append((loc, glob))

    with tile.TileContext(nc) as tc:
        _emit(nc, tc, M, cell_off, C,
              d_x0, d_W1b, d_W2b, d_W3b, d_ipwT, d_ipbC, d_opwTb, d_bvb,
              d_edat, d_out, d_degl, d_degg, ag_bufs)
    nc.compile()
    return nc


def _emit(nc, tc, M, cell_off, C,
          d_x0, d_W1b, d_W2b, d_W3b, d_ipwT, d_ipbC, d_opwTb, d_bvb,
          d_edat, d_out, d_degl, d_degg, ag_bufs):
    from contextlib import ExitStack
    ctx = ExitStack()
    with ctx:
        const = ctx.enter_context(tc.tile_pool(name="const", bufs=1))
        big = ctx.enter_context(tc.tile_pool(name="big", bufs=1))
        scr = ctx.enter_context(tc.tile_pool(name="scr", bufs=2))
        ohp = ctx.enter_context(tc.tile_pool(name="ohp", bufs=8))
        exp_p = ctx.enter_context(tc.tile_pool(name="exp_p", bufs=4))
        tmp = ctx.enter_context(tc.tile_pool(name="tmp", bufs=4))
        ps_gen = ctx.enter_context(tc.tile_pool(name="ps_gen", bufs=2,
                                                space="PSUM"))
        ps_sc = ctx.enter_context(tc.tile_pool(name="ps_sc", bufs=2,
                                               space="PSUM"))
        ps_at = ctx.enter_context(tc.tile_pool(name="ps_at", bufs=2,
                                               space="PSUM"))

        # ---------------- constants ----------------
        iota_i = const.tile([P, P], mybir.dt.int32, name="iota_i")
        nc.gpsimd.iota(iota_i[:], pattern=[[1, P]], base=0, channel_multiplier=0)
        iota_bf = const.tile([P, P], bf16, name="iota_bf")
        nc.vector.tensor_copy(iota_bf[:], iota_i[:])
        ident = const.tile([P, P], bf16, name="ident")
        make_identity(nc, ident[:])
        ones_col = const.tile([P, 1], fp8, name="ones_col")
        nc.vector.memset(ones_col[:], 1.0)
        ones_row_b = const.tile([1, NPC], bf16, name="ones_row_b")
        nc.vector.memset(ones_row_b[:], 1.0)
        ones_row_q = const.tile([1, P], fp8, name="ones_row_q")
        nc.vector.memset(ones_row_q[:], 1.0)
        ones64 = const.tile([1, DH], bf16, name="ones64")
        nc.vector.memset(ones64[:], 1.0)

        # edge chunk data fp32 scalars, one DMA
        edat_sb = const.tile([P, 3 * C], f32, name="edat_sb")
        nc.sync.dma_start(edat_sb[:], d_edat[:, :])
        erow_sb = edat_sb[:, 0:C]
        ecol_sb = edat_sb[:, C:2 * C]
        eww_sb = edat_sb[:, 2 * C:3 * C]

        ipbC = const.tile([P, 6], f32, name="ipbC")
        nc.sync.dma_start(ipbC[:], d_ipbC[:, :])

        def load_cast(dram, rows, cols, tag, dt):
            t_f = scr.tile([P, cols], f32, name="ldf32")
            nc.sync.dma_start(t_f[:rows, :], dram)
            t_b = const.tile([rows, cols], dt, name=tag)
            nc.vector.tensor_copy(t_b[:], t_f[:rows, :])
            return t_b

        W1b = [load_cast(d_W1b[k * P:(k + 1) * P, :], P, HID, f"W1b{k}", bf16)
               for k in range(2)]
        W1b.append(load_cast(d_W1b[2 * P:2 * P + 1, :], 1, HID, "W1b2", bf16))
        W2b = [load_cast(d_W2b[k * P:(k + 1) * P, :], P, HID, f"W2b{k}", bf16)
               for k in range(2)]
        W2b.append(load_cast(d_W2b[2 * P:2 * P + 1, :], 1, HID, "W2b2", bf16))
        W3b = [load_cast(d_W3b[k * P:(k + 1) * P, :], P, DOUT, f"W3b{k}", bf16)
               for k in range(2)]
        W3b.append(load_cast(d_W3b[2 * P:2 * P + 1, :], 1, DOUT, "W3b2", bf16))
        ipwT = [load_cast(d_ipwT[k * P:(k + 1) * P, :], P, 3 * HID,
                          f"ipwT{k}", fp8) for k in range(2)]
        opwTb = [load_cast(d_opwTb[k * P:(k + 1) * P, :], P, HID,
                           f"opwTb{k}", bf16) for k in range(2)]
        opwTb.append(load_cast(d_opwTb[2 * P:2 * P + 1, :], 1, HID,
                               "opwTb2", bf16))
        bvb_q = load_cast(d_bvb[0:1, :], 1, HID, "bvb_q", fp8)

        # x0 fp32 staged n-major: [128, 32*256], 4 DMAs
        x0f = big.tile([P, NST * DIN], f32, name="x0f")
        for q in range(4):
            sl = d_x0[q * (N // 4):(q + 1) * (N // 4), :]
            dst = x0f[:, q * 8 * DIN:(q + 1) * 8 * DIN]
            nc.sync.dma_start(
                dst.rearrange("p (m f) -> p m f", f=DIN),
                sl.rearrange("(m p) f -> p m f", p=P))

        # ---------------- persistent big tiles ----------------
        AT = [big.tile([P, NPC], fp8, name=f"AT{t}") for t in range(NST)]
        x0q = [big.tile([P, DIN], fp8, name=f"x0q{m}") for m in range(NST)]
        xT_full = [big.tile([P, N], fp8, name=f"xTf{k}") for k in range(2)]
        xT_own = [big.tile([P, NPC], fp8, name=f"xTo{k}") for k in range(2)]
        xN_full = big.tile([P, NST * HID], fp8, name="xNf")
        kT = [big.tile([P, N], fp8, name=f"kT{g}") for g in range(2)]
        qT = [big.tile([P, NPC], fp8, name=f"qT{g}") for g in range(2)]
        v_aug = [big.tile([P, NH * (DH + 1)], bf16, name=f"vaug{m}")
                 for m in range(NST)]
        attnT = [big.tile([P, NPC], bf16, name=f"attnT{g}") for g in range(2)]
        x_n = [big.tile([P, HID], bf16, name=f"x_n{m}") for m in range(NSTRIP)]
        xq_n = [big.tile([P, HID], fp8, name=f"xq_n{m}")
                for m in range(NSTRIP)]
        agg = [big.tile([P, HID], bf16, name=f"agg{m}") for m in range(NSTRIP)]
        aggT = [big.tile([P, NPC], bf16, name=f"aggT{k}") for k in range(2)]
        den_r = const.tile([1, NH * NPC], f32, name="den_r")
        rden_f = const.tile([1, NH * NPC], f32, name="rden_f")
        rden_r = const.tile([1, NH * NPC], bf16, name="rden_r")

        # ---------------- phase 1: build A^T (raw weights, fp8) ------------
        for s in range(NSTRIP):
            for t in range(NST):
                m = int(M[s, t])
                dst = AT[t][:, s * P:(s + 1) * P]
                if m == 0:
                    nc.vector.memset(dst, 0.0)
                    continue
                pA = ps_gen.tile([P, P], f32, name="ps_mm")
                for j in range(m):
                    o = int(cell_off[s, t]) + j
                    roh = ohp.tile([P, P], bf16, name="roh")
                    coh = ohp.tile([P, P], bf16, name="coh")
                    nc.vector.tensor_scalar(
                        roh[:], iota_bf[:], erow_sb[:, o:o + 1],
                        eww_sb[:, o:o + 1], op0=ALU.is_equal, op1=ALU.mult)
                    nc.vector.tensor_scalar(
                        coh[:], iota_bf[:], ecol_sb[:, o:o + 1], None,
                        op0=ALU.is_equal)
                    nc.tensor.matmul(pA[:], lhsT=roh[:], rhs=coh[:],
                                     start=(j == 0), stop=(j == m - 1))
                nc.scalar.copy(dst, pA[:])

        # ---------------- phase 2: degree + dinv vectors --------------------
        deg_own = const.tile([P, NSTRIP], f32, name="deg_own")
        for s in range(NSTRIP):
            pd = ps_gen.tile([P, 1], f32, name="ps_mm")
            for t in range(NST):
                nc.tensor.matmul(pd[:], lhsT=AT[t][:, s * P:(s + 1) * P],
                                 rhs=ones_col[:], start=(t == 0),
                                 stop=(t == NST - 1))
            nc.scalar.copy(deg_own[:, s:s + 1], pd[:])
        dinv_own = const.tile([P, NSTRIP], f32, name="dinv_own")
        nc.scalar.sqrt(dinv_own[:], deg_own[:])
        nc.vector.reciprocal(dinv_own[:], dinv_own[:])
        dinv_own_c = const.tile([P, NSTRIP], f32, name="dinv_own_c")
        nc.vector.tensor_scalar(dinv_own_c[:], dinv_own[:], 1.0 / RSCALE,
                                None, op0=ALU.mult)
        dinv_own_r = const.tile([P, NSTRIP], f32, name="dinv_own_r")
        nc.vector.tensor_scalar(dinv_own_r[:], dinv_own[:], RSCALE,
                                None, op0=ALU.mult)

        nc.sync.dma_start(
            d_degl.ap().rearrange("(m p) -> p m", p=P), deg_own[:])
        nc.gpsimd.collective_compute(
            "AllGather", ALU.bypass, replica_groups=RG,
            ins=[d_degl[:]], outs=[d_degg[:]])
        deg_all = const.tile([P, NST], f32, name="deg_all")
        nc.sync.dma_start(deg_all[:],
                          d_degg.ap().rearrange("(t p) -> p t", p=P))
        dinv_all = const.tile([P, NST], f32, name="dinv_all")
        nc.scalar.sqrt(dinv_all[:], deg_all[:])
        nc.vector.reciprocal(dinv_all[:], dinv_all[:])

        # x~0 = dinv[src] * x0, fp8
        for m_ in range(NST):
            nc.scalar.activation(
                x0q[m_][:], x0f[:, m_ * DIN:(m_ + 1) * DIN], AF.Identity,
                scale=dinv_all[:, m_:m_ + 1])

        # ---------------- helpers ----------------
        def transpose_128(dst_ap, src_ap):
            pT = ps_gen.tile([P, P], bf16, name="ps_tr")
            nc.tensor.transpose(pT[:], src_ap, ident[:])
            nc.scalar.copy(dst_ap, pT[:])

        def aggregate(rhs_fn, width, dinv_vec):
            """agg[mm] = dinv_vec[mm] * (sum_t AT[t](slice mm) @ rhs(t))."""
            for mm in range(NSTRIP):
                pg = ps_gen.tile([P, width], f32, name="ps_mm")
                for t in range(NST):
                    nc.tensor.matmul(pg[:], lhsT=AT[t][:, mm * P:(mm + 1) * P],
                                     rhs=rhs_fn(t),
                                     start=(t == 0), stop=(t == NST - 1))
                nc.scalar.activation(agg[mm][:, :width], pg[:], AF.Identity,
                                     scale=dinv_vec[:, mm:mm + 1])

        def dense_out(lhsT_tiles, rhs3, width, evict):
            """psum = sum_k lhsT[k].T @ rhs3[k] + ones-row @ rhs3[2] (bias)."""
            for mm in range(NSTRIP):
                po = ps_gen.tile([P, width], f32, name="ps_mm")
                for k in range(2):
                    nc.tensor.matmul(
                        po[:], lhsT=lhsT_tiles[k][:, mm * P:(mm + 1) * P],
                        rhs=rhs3[k][:, :width], start=(k == 0), stop=False)
                nc.tensor.matmul(po[:], lhsT=ones_row_b[0:1, mm * P:(mm + 1) * P],
                                 rhs=rhs3[2][:, :width], start=False, stop=True)
                evict(mm, po[:])

        def gcn_dense(W3, width, evict):
            """agg (bf16, n-major) -> transpose -> dense W3 -> evict."""
            for mm in range(NSTRIP):
                for k in range(2):
                    transpose_128(aggT[k][:, mm * P:(mm + 1) * P],
                                  agg[mm][:, k * P:(k + 1) * P])
            dense_out(aggT, W3, width, evict)

        def x_to_own_T():
            for mm in range(NSTRIP):
                for k in range(2):
                    transpose_128(xT_own[k][:, mm * P:(mm + 1) * P],
                                  x_n[mm][:, k * P:(k + 1) * P])

        def compute_qT():
            for g in range(2):
                pq = ps_gen.tile([P, NPC], f32, name="ps_mm")
                for k in range(2):
                    nc.tensor.matmul(pq[:], lhsT=ipwT[k][:, g * P:(g + 1) * P],
                                     rhs=xT_own[k][:], start=(k == 0),
                                     stop=(k == 1))
                nc.scalar.activation(qT[g][:], pq[:], AF.Identity,
                                     bias=ipbC[:, g:g + 1])

        def ag_fmajor(ag_idx):
            loc, glob = ag_bufs[ag_idx]
            for k in range(2):
                nc.sync.dma_start(loc[k * P:(k + 1) * P, :], xT_own[k][:])
            nc.gpsimd.collective_compute(
                "AllGather", ALU.bypass, replica_groups=RG,
                ins=[loc[:, :]], outs=[glob[:, :, :]])
            for k in range(2):
                nc.sync.dma_start(
                    xT_full[k][:].rearrange("p (c n) -> p c n", n=NPC),
                    glob[:, k * P:(k + 1) * P, :].rearrange("c p n -> p c n"))

        def ag_nmajor(ag_idx):
            loc, glob = ag_bufs[ag_idx]
            for mm in range(NSTRIP):
                nc.sync.dma_start(loc[mm * P:(mm + 1) * P, :], xq_n[mm][:])
            nc.gpsimd.collective_compute(
                "AllGather", ALU.bypass, replica_groups=RG,
                ins=[loc[:, :]], outs=[glob[:, :, :]])
            for c in range(NCORES):
                dst = xN_full[:, c * NSTRIP * HID:(c + 1) * NSTRIP * HID]
                nc.sync.dma_start(
                    dst.rearrange("p (m h) -> p m h", h=HID),
                    glob[c, :, :].rearrange("(m p) h -> p m h", p=P))

        # ---------------- MHA ----------------
        def mha(out_evict):
            # kT (all nodes), 2 head-groups; evict on DVE (bias add)
            for g in range(2):
                for n in range(NCORES):
                    pk = ps_gen.tile([P, NPC], f32, name="ps_mm")
                    for k in range(2):
                        nc.tensor.matmul(
                            pk[:],
                            lhsT=ipwT[k][:, HID + g * P:HID + (g + 1) * P],
                            rhs=xT_full[k][:, n * NPC:(n + 1) * NPC],
                            start=(k == 0), stop=(k == 1))
                    nc.vector.tensor_scalar(
                        kT[g][:, n * NPC:(n + 1) * NPC], pk[:],
                        ipbC[:, 2 + g:3 + g], None, op0=ALU.add)
            # v (n-major, all nodes), bias via ones-row MM; ACT evict
            for m_ in range(NST):
                pv = ps_gen.tile([P, HID], f32, name="ps_mm")
                for k in range(2):
                    nc.tensor.matmul(pv[:],
                                     lhsT=xT_full[k][:, m_ * P:(m_ + 1) * P],
                                     rhs=ipwT[k][:, 2 * HID:3 * HID],
                                     start=(k == 0), stop=False)
                nc.tensor.matmul(pv[:], lhsT=ones_row_q[0:1, :],
                                 rhs=bvb_q[:], start=False, stop=True)
                va = v_aug[m_][:].rearrange("p (h x) -> p h x", x=DH + 1)
                nc.scalar.copy(va[:, :, 0:DH],
                               pv[:].rearrange("p (h x) -> p h x", x=DH))
                nc.vector.memset(va[:, :, DH:DH + 1], 1.0)
            # attention per head: scores -> exp -> AV(+den)
            for h in range(NH):
                g, r = h // 2, (h % 2) * DH
                pat = ps_at.tile([DH + 1, NPC], f32, name="ps_at")
                for m_ in range(NST):
                    psc = ps_sc.tile([P, NPC], f32, name="ps_sc")
                    nc.tensor.matmul(psc[:],
                                     lhsT=kT[g][r:r + DH, m_ * P:(m_ + 1) * P],
                                     rhs=qT[g][r:r + DH, :],
                                     start=True, stop=True)
                    et = exp_p.tile([P, NPC], bf16, name="expT")
                    nc.scalar.activation(et[:], psc[:], AF.Exp,
                                         scale=float(1.0 / np.sqrt(DH)))
                    nc.tensor.matmul(
                        pat[:],
                        lhsT=v_aug[m_][:, h * (DH + 1):(h + 1) * (DH + 1)],
                        rhs=et[:], start=(m_ == 0), stop=(m_ == NST - 1))
                nc.scalar.copy(attnT[g][r:r + DH, :], pat[0:DH, :])
                nc.scalar.copy(den_r[0:1, h * NPC:(h + 1) * NPC],
                               pat[DH:DH + 1, :])
            # denominator reciprocal (approx is plenty: den ~ 4096, smooth)
            nc.vector.reciprocal_approx_fast(rden_f[:], den_r[:])
            nc.vector.tensor_copy(rden_r[:], rden_f[:])
            for h in range(NH):
                g, r = h // 2, (h % 2) * DH
                pb = ps_sc.tile([DH, NPC], f32, name="ps_sc")
                nc.tensor.matmul(pb[:], lhsT=ones64[0:1, :],
                                 rhs=rden_r[0:1, h * NPC:(h + 1) * NPC],
                                 start=True, stop=True)
                nc.vector.tensor_tensor(attnT[g][r:r + DH, :],
                                        attnT[g][r:r + DH, :], pb[:],
                                        op=ALU.mult)
            # out-proj (+bias via K-aug ones-row) -> evict
            dense_out(attnT, opwTb, HID, out_evict)

        # ---------------- phase 3: GCN1 (aggregate-first, x0 local) --------
        aggregate(lambda t: x0q[t][:], DIN, dinv_own)
        def evict_x1(mm, ps):
            nc.scalar.copy(x_n[mm][:], ps)
        gcn_dense(W1b, HID, evict_x1)
        x_to_own_T()
        compute_qT()
        ag_fmajor(0)

        # ---------------- phase 4: MHA1 -> x2 (relu * dinv * RSCALE) -------
        def evict_relu_dinv(mm, ps):
            nc.scalar.activation(xq_n[mm][:], ps, AF.Relu,
                                 scale=dinv_own_r[:, mm:mm + 1])
        mha(evict_relu_dinv)
        ag_nmajor(1)

        # ---------------- phase 5: GCN2 ----------------
        aggregate(lambda t: xN_full[:, t * HID:(t + 1) * HID], HID,
                  dinv_own_c)
        gcn_dense(W2b, HID, evict_x1)
        x_to_own_T()
        compute_qT()
        ag_fmajor(2)

        # ---------------- phase 6: MHA2 -> x4 ----------------
        mha(evict_relu_dinv)
        ag_nmajor(3)

        # ---------------- phase 7: GCN3 + sigmoid ----------------
        aggregate(lambda t: xN_full[:, t * HID:(t + 1) * HID], HID,
                  dinv_own_c)
        def evict_sigmoid(mm, ps):
            o_sb = tmp.tile([P, DOUT], f32, name="o_sb")
            nc.scalar.activation(o_sb[:], ps, AF.Sigmoid)
            nc.sync.dma_start(d_out[mm * P:(mm + 1) * P, :], o_sb[:])
        gcn_dense(W3b, DOUT, evict_sigmoid)


# ----------------------------------------------------------------------------
# Entry point
# ----------------------------------------------------------------------------

_CACHE = {}
TRACE = False
LAST_RESULTS = None


def _get_program(M, cell_off):
    key = (M.tobytes(), cell_off.tobytes())
    if key not in _CACHE:
        _CACHE[key] = _build_program(M, cell_off)
    return _CACHE[key]


def make_in_maps(node_features, edge_index, edge_weight, W1, b1, W2, b2, W3,
                 b3, in_proj_w, in_proj_b, out_proj_w, out_proj_b):
    M, cell_off, edat = _prep_edges(edge_index, edge_weight)
    asf = lambda a: np.ascontiguousarray(a, np.float32)
    common = {
        "x0": asf(node_features),
        "W1b": asf(np.vstack([W1, b1[None, :]])),
        "W2b": asf(np.vstack([W2, b2[None, :]])),
        "W3b": asf(np.vstack([W3, b3[None, :]])),
        "ipwT": asf(np.asarray(in_proj_w, np.float32).T),
        "ipbC": asf(np.asarray(in_proj_b, np.float32).reshape(6, P).T),
        "opwTb": asf(np.vstack([np.asarray(out_proj_w, np.float32).T,
                                out_proj_b[None, :]])),
        "bvb": asf(np.asarray(in_proj_b, np.float32)[None, 2 * HID:3 * HID]),
    }
    in_maps = []
    for c in range(NCORES):
        m = dict(common)
        m["edat"] = np.ascontiguousarray(edat[c])
        in_maps.append(m)
    return M, cell_off, in_maps


def kernel(**inputs):
    global LAST_RESULTS
    inputs = {k: np.asarray(v) for k, v in inputs.items()}
    M, cell_off, in_maps = make_in_maps(**inputs)
    nc = _get_program(M, cell_off)
    res = bass_utils.run_bass_kernel_spmd(nc, in_maps,
                                          core_ids=list(range(NCORES)),
                                          trace=TRACE)
    LAST_RESULTS = res
    out = np.concatenate([res.results[c]["out"] for c in range(NCORES)],
                         axis=0)
    return out.astype(np.float32)
